# revision 62
# baseline (speedup 1.0000x reference)
"""GCN (2-layer, GCNConv+BN+ReLU) on 8 TRN2 NeuronCores via Bass.

Host plan: permute nodes so that pass p (= original src quarter) occupies
permuted rows [p*QROWS, (p+1)*QROWS); within each quarter the nodes are dealt
to 2 cores x WPC windows of 32 nodes, balancing in-degree. Edges are
dst-sharded; per (window, pass) chunk sizes form a COMMON schedule across
cores (SPMD: one program, per-core data).

Device per layer:
  t = h @ W (PE)  -> fp16 table shard -> AllGather -> full table
  4 gather streams (dma_gather, 2048 idx/call, 4 queues) -> msg tiles
  one DVE broadcast multiply folds the edge weight into each gathered tile
  staircase matmuls: S_onehot(fp8)[128s, 32win].T @ msg -> PSUM quad accumulate
  flush -> h_pre (fp16); BN stats (+AllReduce) -> normalize+ReLU
"""
import sys
sys.path.insert(0, '/opt/trn_rl_repo')

import numpy as np
import concourse.bass as bass
import concourse.bacc as bacc
import concourse.tile as tile
from concourse import mybir
from concourse.ap import AP
from contextlib import ExitStack

FP32 = mybir.dt.float32
FP16 = mybir.dt.float16
F8 = mybir.dt.float8e4
I16 = mybir.dt.int16

GSLOTS = 1024                 # slots per gather call (8 sub-tiles of 128)
GSUB = GSLOTS // 128


class Plan:
    pass


def build_plan(edge_index, edge_weight, N, D_IN, HID, EPS, n_cores=8, n_pass=4, win=32, seed=0):
    """Host-side schedule. Returns Plan with per-core tensors + common schedule.

    Table layout is pass-major: pass p (a contiguous range of quads in every
    core's shard) occupies table rows [tbase[p], tbase[p] + n_cores*crows[p]),
    core-major within the pass. Each pass's AllGather output is contiguous and
    its subtable is < 32768 rows (int16 gather indices)."""
    p = Plan()
    src = np.asarray(edge_index[0], dtype=np.int64)
    dst = np.asarray(edge_index[1], dtype=np.int64)
    w = np.asarray(edge_weight, dtype=np.float32)
    E = src.shape[0]

    nsh = ((N + n_cores - 1) // n_cores + 127) // 128 * 128   # 12544
    ntot = nsh * n_cores
    wpc = nsh // win                            # windows per core 392
    quads = nsh // 128                          # 98

    # quad -> pass chunks. Mildly uneven: a smaller pass 0 lets the first
    # AllGather chunk (and the first gathers) fire earlier without starving
    # the pair-0 gather stream the way a tiny pass 0 does.
    # Cap: 8 cores * 128 * nq <= 32768 rows per pass (int16 gather indices).
    if n_pass == 4 and quads == 98:
        qb = [20, 45, 71, 98]
    else:
        qb = [((c + 1) * quads) // n_pass for c in range(n_pass)]    # cumulative
    qstart = [0] + qb[:-1]
    nquads_p = [qb[i] - qstart[i] for i in range(n_pass)]
    crows = [128 * nq for nq in nquads_p]       # rows per core per pass
    tbase = np.concatenate([[0], np.cumsum([n_cores * r for r in crows])]).astype(np.int64)
    pass_of_quad = np.zeros(quads, dtype=np.int64)
    for i in range(n_pass):
        pass_of_quad[qstart[i]:qb[i]] = i
    p.qstart, p.crows, p.tbase = qstart, crows, tbase

    # ---- node -> (core, window, pos) balanced assignment (global snake) ----
    indeg = np.bincount(dst, minlength=N)
    order = np.argsort(-indeg, kind="stable")
    nbins = n_cores * wpc
    padded = np.full(nbins * win, -1, dtype=np.int64)
    padded[:N] = order
    grid = padded.reshape(win, nbins)
    for r in range(1, win, 2):
        grid[r] = grid[r][::-1]
    # bin b -> core b % n_cores, window b // n_cores
    bcore = np.arange(nbins) % n_cores
    bwin = np.arange(nbins) // n_cores
    # shard row (within core) of each grid cell
    cell_core = np.broadcast_to(bcore, (win, nbins))
    nodes = grid.reshape(-1)
    cc = cell_core.reshape(-1)
    valid = nodes >= 0

    def placement(wslot):
        slot_vec = wslot[bcore, bwin]
        cell_row = np.broadcast_to(slot_vec * win, (win, nbins)) \
            + np.arange(win)[:, None]
        cr = cell_row.reshape(-1)
        row_of = np.full(N, -1, dtype=np.int64)
        row_of[nodes[valid]] = cc[valid] * nsh + cr[valid]
        perm_of = np.full(ntot, -1, dtype=np.int64)
        perm_of[cc[valid] * nsh + cr[valid]] = nodes[valid]
        return row_of, perm_of

    # table row of a global shard row
    def table_row_of(grow):
        core = grow // nsh
        srow = grow % nsh
        qd = srow // 128
        pp = pass_of_quad[qd]
        return (tbase[pp] + core * np.asarray(crows)[pp]
                + (srow - 128 * np.asarray(qstart)[pp])), pp

    # ---- edges to permuted space ----
    def edge_stats(row_of):
        gdst = row_of[dst]
        gsrc = row_of[src]
        tsrc, epass = table_row_of(gsrc)
        lidx = tsrc - tbase[epass]
        assert lidx.max() < 32768
        ecore = gdst // nsh
        ewin = (gdst % nsh) // win
        key = (ecore * wpc + ewin) * n_pass + epass
        cnt = np.bincount(key, minlength=n_cores * wpc * n_pass) \
            .reshape(n_cores, wpc, n_pass)
        return gdst, epass, lidx, ecore, ewin, cnt

    # phase 1: identity placement, measure per-(window, pass) counts
    wslot = np.broadcast_to(np.arange(wpc), (n_cores, wpc)).copy()
    row_of, perm_of = placement(wslot)
    _, _, _, _, _, cnt = edge_stats(row_of)
    # phase 2: within each pass region, permute windows per core so count
    # vectors align across cores (schedule is max-over-cores). Window moves
    # inside a region keep every node's src-pass, so counts just permute.
    try:
        from scipy.optimize import linear_sum_assignment
        for r in range(n_pass):
            s0, s1 = 4 * qstart[r], 4 * qb[r]
            env = cnt[0, s0:s1].astype(np.float64)
            for c in range(1, n_cores):
                C = np.maximum(env[:, None, :], cnt[c, s0:s1][None, :, :]).sum(2) \
                    - env.sum(1)[:, None]
                ri, ci = linear_sum_assignment(C)
                wslot[c, s0 + ci] = s0 + np.arange(s1 - s0)
                env = np.maximum(env, cnt[c, s0:s1][ci])
        row_of, perm_of = placement(wslot)
    except Exception:
        pass
    gdst, epass, lidx, ecore, ewin, cnt = edge_stats(row_of)
    gsrc = row_of[src]
    sched = cnt.max(axis=0)             # [wpc, n_pass] common chunk sizes
    stream_len = sched.sum(axis=0)
    for pp in range(n_pass):
        extra = (-stream_len[pp]) % 128
        sched[wpc - 1, pp] += extra
    stream_len = sched.sum(axis=0)
    p.stream_len = stream_len.astype(np.int64)

    chunk_off = np.zeros((wpc + 1, n_pass), dtype=np.int64)
    chunk_off[1:] = np.cumsum(sched, axis=0)

    # ---- per-core slot arrays ----
    idx_arr = np.zeros((n_cores, n_pass, int(stream_len.max())), dtype=np.int16)
    w_arr = np.zeros((n_cores, n_pass, int(stream_len.max())), dtype=np.float32)
    col_arr = np.zeros((n_cores, n_pass, int(stream_len.max())), dtype=np.int32)
    eorder = np.lexsort((lidx, epass, ewin, ecore))
    po = epass[eorder]; co = ecore[eorder]
    wo = ewin[eorder]; li = lidx[eorder]; ww = w[eorder]
    gdo = gdst[eorder]
    ekey = (co * wpc + wo) * n_pass + po
    grp_start = np.zeros(n_cores * wpc * n_pass + 1, dtype=np.int64)
    grp_start[1:] = np.cumsum(cnt.reshape(-1))
    pos_in_grp = np.arange(E) - grp_start[ekey]
    slot = chunk_off[wo, po] + pos_in_grp
    idx_arr[co, po, slot] = li.astype(np.int16)
    w_arr[co, po, slot] = ww
    col_arr[co, po, slot] = (gdo % nsh) % 128

    p.N, p.E, p.n_cores, p.n_pass = N, E, n_cores, n_pass
    p.D_IN, p.HID, p.EPS = D_IN, HID, EPS
    p.nsh, p.ntot, p.win, p.wpc, p.quads = nsh, ntot, win, wpc, quads
    p.perm_of, p.row_of = perm_of, row_of
    p.sched, p.chunk_off = sched, chunk_off

    # layer-1 pass-0 table is computed locally on every core (x is cheap to
    # replicate for those rows) so the first gathers don't wait on AllGather.
    # x0_nodes[t] = original node id of table1[0] row t (or -1 for padding).
    crows0 = crows[0]
    x0_nodes = np.full(n_cores * crows0, -1, dtype=np.int64)
    for c in range(n_cores):
        rows = perm_of[c * nsh:c * nsh + crows0]
        x0_nodes[c * crows0:(c + 1) * crows0] = rows
    p.x0_nodes = x0_nodes
    p.crows0 = crows0

    # ---- matmul schedule (common, pass-PAIR-major) ----
    # Processing pass pairs (0,1) then (2,3) lets aggregation start as soon
    # as the first AllGather chunks land; PSUM accumulates within a pair and
    # partial sums are added into SBUF between pairs.
    mm = []
    s_count = 0
    n_pair = n_pass // 2
    for pg in range(n_pair):
        for q in range(quads):
            for wi in range(4):
                wdx = q * 4 + wi
                group_has_op = False
                for pp in (2 * pg, 2 * pg + 1):
                    a = int(chunk_off[wdx, pp]); b = int(chunk_off[wdx + 1, pp])
                    if b == a:
                        continue
                    group_has_op = True
                    t0, t1 = a // 128, (b - 1) // 128
                    for t in range(t0, t1 + 1):
                        mm.append(dict(q=q, wi=wi, p=pp, pg=pg, t=t, s=s_count,
                                       lo=max(a, t * 128), hi=min(b, (t + 1) * 128)))
                        s_count += 1
                if not group_has_op:
                    # zero-S placeholder so the PSUM strip gets start+stop
                    mm.append(dict(q=q, wi=wi, p=2 * pg, pg=pg, t=0, s=s_count,
                                   lo=0, hi=0))
                    s_count += 1
    p.n_pair = n_pair

    # ---- S one-hot fill (vectorized) + empty-op pruning ----
    op_lo = np.array([op["lo"] for op in mm])
    op_hi = np.array([op["hi"] for op in mm])
    op_p = np.array([op["p"] for op in mm])
    op_t = np.array([op["t"] for op in mm])
    op_wi = np.array([op["wi"] for op in mm])
    lens = op_hi - op_lo
    opidx = np.repeat(np.arange(s_count), lens)
    sl = np.concatenate([np.arange(a, b) for a, b in zip(op_lo, op_hi)]) if s_count else np.array([], np.int64)
    rows = sl - op_t[opidx] * 128
    # usage per (core, entry): valid one-hot hits
    ent_core = []
    ent_rows = []
    ent_cols = []
    ent_op = []
    op_used = np.zeros(s_count, dtype=bool)
    for c in range(n_cores):
        cols = col_arr[c, op_p[opidx], sl] - op_wi[opidx] * 32
        wv = w_arr[c, op_p[opidx], sl]
        m = (cols >= 0) & (cols < 32) & (wv != 0)
        ent_core.append(np.full(m.sum(), c))
        ent_rows.append(rows[m])
        ent_cols.append(cols[m])
        ent_op.append(opidx[m])
        np.logical_or.at(op_used, opidx[m], True)

    # keep at least one op per (q, wi, pair) group so every PSUM strip gets
    # start+stop within each pair phase
    group_of = {}
    for i, op in enumerate(mm):
        group_of.setdefault((op["q"], op["wi"], op["pg"]), []).append(i)
    for g, ops_in_g in group_of.items():
        if not op_used[ops_in_g].any():
            op_used[ops_in_g[0]] = True

    old2new = np.full(s_count, -1, dtype=np.int64)
    old2new[op_used] = np.arange(op_used.sum())
    mm = [dict(op, s=int(old2new[i])) for i, op in enumerate(mm) if op_used[i]]
    p.mm, p.n_mm = mm, int(op_used.sum())

    first_op = {}
    last_op = {}
    for i, op in enumerate(mm):
        k = (op["q"], op["wi"], op["pg"])
        if k not in first_op:
            first_op[k] = i
        last_op[k] = i
    p.first_op = {v: True for v in first_op.values()}
    p.last_op = {v: True for v in last_op.values()}

    S = np.zeros((n_cores, 128, 32 * p.n_mm), dtype=mybir.dt.np(F8))
    one = np.ones(1, dtype=mybir.dt.np(F8))[0]
    for c in range(n_cores):
        so = old2new[ent_op[c]]
        S[c, ent_rows[c], so * 32 + ent_cols[c]] = one
    p.S = S

    # w per slot, gather-tile aligned: [128, nst_total] fp16
    nst_per_pass = [int(stream_len[pp]) // 128 for pp in range(n_pass)]
    p.sub_base = np.concatenate([[0], np.cumsum(nst_per_pass)]).astype(np.int64)
    nst_total = int(p.sub_base[-1])
    p.nst_total = nst_total
    wt = np.zeros((n_cores, 128, nst_total, 2), dtype=np.float16)
    for c in range(n_cores):
        for pp in range(n_pass):
            L = int(stream_len[pp])
            v = w_arr[c, pp, :L].reshape(-1, 128).T.astype(np.float16)
            wt[c, :, p.sub_base[pp]:p.sub_base[pp + 1], 0] = v
            wt[c, :, p.sub_base[pp]:p.sub_base[pp + 1], 1] = v
    p.w_tiles = wt

    p.idx_wrapped = []
    for pp in range(n_pass):
        L = int(stream_len[pp])
        a = idx_arr[:, pp, :L]
        wr = np.stack([np.tile(a[c].reshape(-1, 16).T, (8, 1)) for c in range(n_cores)])
        p.idx_wrapped.append(np.ascontiguousarray(wr))

    p.gathers = []
    for pp in range(n_pass):
        L = int(stream_len[pp])
        assert L % 128 == 0
        p.gathers.append([(o, min(GSLOTS, L - o))
                          for o in range(0, L, GSLOTS)])
    return p


def build_nc(p, num_bufs=3):
    """Build the bass program from the common schedule."""
    n_pass, nsh, quads = p.n_pass, p.nsh, p.quads
    D_IN, HID = p.D_IN, p.HID
    din_t = D_IN // 128
    N = p.N

    nc = bacc.Bacc("TRN2", debug=False, num_devices=p.n_cores, num_swdge_queues=4)
    xT_in = nc.dram_tensor("xT", [D_IN, nsh], FP16, kind="ExternalInput")
    W1_in = nc.dram_tensor("W1", [D_IN, HID], FP16, kind="ExternalInput")
    W2_in = nc.dram_tensor("W2", [HID, HID], FP16, kind="ExternalInput")
    g1_in = nc.dram_tensor("g1", [1, HID], FP32, kind="ExternalInput")
    be1_in = nc.dram_tensor("be1", [1, HID], FP32, kind="ExternalInput")
    g2_in = nc.dram_tensor("g2", [1, HID], FP32, kind="ExternalInput")
    be2_in = nc.dram_tensor("be2", [1, HID], FP32, kind="ExternalInput")
    ident_in = nc.dram_tensor("ident", [128, 128], FP16, kind="ExternalInput")
    S_in = nc.dram_tensor("S", [128, 32 * p.n_mm], F8, kind="ExternalInput")
    wt_in = nc.dram_tensor("wt", [128, p.nst_total, 2], FP16, kind="ExternalInput")
    idx_ins = [nc.dram_tensor(f"idx{pp}", [128, int(p.stream_len[pp]) // 16], I16,
                              kind="ExternalInput") for pp in range(n_pass)]
    out_t = nc.dram_tensor("out", [nsh, HID], FP16, kind="ExternalOutput")

    groups = [list(range(p.n_cores))]

    with tile.TileContext(nc) as tc, ExitStack() as ctx:
        dram = ctx.enter_context(tc.tile_pool(name="dram", bufs=1, space="DRAM"))
        const = ctx.enter_context(tc.tile_pool(name="const", bufs=1))
        persist = ctx.enter_context(tc.tile_pool(name="persist", bufs=1))
        small = ctx.enter_context(tc.tile_pool(name="small", bufs=2))
        tpsum = ctx.enter_context(tc.tile_pool(name="tpsum", bufs=1, space="PSUM"))
        apsum = ctx.enter_context(tc.tile_pool(name="apsum", bufs=4, space="PSUM"))
        gpools = [ctx.enter_context(tc.tile_pool(name=f"gath{i}", bufs=8))
                  for i in range(n_pass)]
        ipools = [ctx.enter_context(tc.tile_pool(name=f"idxp{i}", bufs=2))
                  for i in range(n_pass)]
        spool = ctx.enter_context(tc.tile_pool(name="spool", bufs=4))
        fpool = ctx.enter_context(tc.tile_pool(name="flush", bufs=4))

        # resident xT first (needed by the very first matmuls)
        xT_sb = const.tile([128, din_t, nsh], FP16)
        XCH = 16
        xcw = nsh // XCH
        for i in range(XCH):
            eng = [nc.sync, nc.scalar][i % 2]
            eng.dma_start(
                xT_sb[:, :, i * xcw:(i + 1) * xcw],
                xT_in.ap()[:, i * xcw:(i + 1) * xcw]
                .rearrange("(a b) c -> b a c", b=128))
        W1_sb = const.tile([128, din_t, HID], FP16)
        nc.sync.dma_start(W1_sb[:], W1_in.ap().rearrange("(a b) c -> b a c", b=128))
        W2_sb = const.tile([128, HID], FP16)
        nc.scalar.dma_start(W2_sb[:], W2_in.ap())
        ident = const.tile([128, 128], FP16)
        nc.sync.dma_start(ident[:], ident_in.ap())
        ones_sb = const.tile([128, 1], FP32)
        nc.vector.memset(ones_sb[:], 1.0)
        gb_sb = const.tile([1, 4, HID], FP32)
        nc.sync.dma_start(gb_sb[:, 0, :], g1_in.ap())
        nc.sync.dma_start(gb_sb[:, 1, :], be1_in.ap())
        nc.sync.dma_start(gb_sb[:, 2, :], g2_in.ap())
        nc.sync.dma_start(gb_sb[:, 3, :], be2_in.ap())
        wt_sb = const.tile([128, p.nst_total, 2], FP16)
        nc.scalar.dma_start(wt_sb[:], wt_in.ap())

        # per-pass tiles: a single tile would serialize on coarse
        # write-after-read dependencies (pass p's AllGather read blocks
        # pass p+1's stores; layer-2 flushes block on producer2 reads).
        npq = [p.qstart[i + 1] - p.qstart[i] if i + 1 < n_pass
               else quads - p.qstart[i] for i in range(n_pass)]
        h_pre = [persist.tile([128, npq[i], HID], FP16, name=f"hpre{i}")
                 for i in range(n_pass)]

        def hq(q):
            for i in reversed(range(n_pass)):
                if q >= p.qstart[i]:
                    return h_pre[i][:, q - p.qstart[i], :]

        def pass_of(q):
            for i in reversed(range(n_pass)):
                if q >= p.qstart[i]:
                    return i

        # DRAM staging (per pass)
        shard1 = [dram.tile([p.crows[i], HID], FP16, name=f"sh1_{i}")
                  for i in range(n_pass)]
        shard2 = [dram.tile([p.crows[i], HID], FP16, name=f"sh2_{i}")
                  for i in range(n_pass)]
        table1 = [dram.tile([p.n_cores * p.crows[i], HID], FP16,
                            addr_space="Shared", name=f"tab1_{i}")
                  for i in range(n_pass)]
        table2 = [dram.tile([p.n_cores * p.crows[i], HID], FP16,
                            addr_space="Shared", name=f"tab2_{i}")
                  for i in range(n_pass)]

        def make_producer1(chunk_done):
            # t1 = x @ W1 from the SBUF-resident xT; cast+store on scalar
            def produce_quad(t):
                ps = tpsum.tile([128, HID], FP32, tag="mmq", bufs=2)
                for k in range(din_t):
                    nc.tensor.matmul(ps[:], xT_sb[:, k, t * 128:(t + 1) * 128],
                                     W1_sb[:, k, :],
                                     start=(k == 0), stop=(k == din_t - 1))
                o16 = fpool.tile([128, HID], FP16, tag="o16")
                nc.scalar.activation(o16[:], ps[:],
                                     mybir.ActivationFunctionType.Copy)
                i = pass_of(t)
                off = (t - p.qstart[i]) * 128
                eng = [nc.sync, nc.scalar][t % 2]
                eng.dma_start(shard1[i][off:off + 128, :], o16[:])
                chunk_done(t, shard1)
            return produce_quad

        SCHUNK = 64
        IB = 4                       # gather calls per idx-load batch

        def mk_agg(table):
            """Per-layer aggregation state: prefetch() then run(layer)."""
            st = {"gmap": {}, "imap": {}, "s_tiles": {}, "qrr": 0}

            def ensure_idx(pp, bi):
                if (pp, bi) in st["imap"]:
                    return st["imap"][(pp, bi)]
                lo = bi * IB * GSLOTS
                if lo >= int(p.stream_len[pp]):
                    return None
                it = ipools[pp].tile([128, IB * GSLOTS // 16], I16, tag=f"idx{pp}")
                hi = min(int(p.stream_len[pp]), lo + IB * GSLOTS)
                nc.sync.dma_start(it[:, :(hi - lo) // 16],
                                  idx_ins[pp].ap()[:, lo // 16:hi // 16])
                st["imap"][(pp, bi)] = it
                return it

            def s_load(blk):
                if blk in st["s_tiles"] or blk * SCHUNK >= p.n_mm:
                    return
                stile = spool.tile([128, SCHUNK * 32], F8, tag="S")
                lo = blk * SCHUNK * 32
                hi = min(32 * p.n_mm, lo + SCHUNK * 32)
                nc.sync.dma_start(stile[:, :hi - lo], S_in.ap()[:, lo:hi])
                st["s_tiles"][blk] = stile

            def emit_gather(pp, gi):
                gmap = st["gmap"]
                if (pp, gi) in gmap or gi >= len(p.gathers[pp]):
                    return
                off, cnt = p.gathers[pp][gi]
                it = ensure_idx(pp, gi // IB)
                ensure_idx(pp, gi // IB + 1)
                i0 = (gi % IB) * (GSLOTS // 16)
                gt = gpools[pp].tile([128, GSUB, HID], FP16, tag=f"g{pp}")
                nc.gpsimd.dma_gather(
                    gt[:, :cnt // 128, :],
                    table[pp][:, :],
                    it[:, i0:i0 + cnt // 16],
                    num_idxs=cnt, num_idxs_reg=cnt, elem_size=HID,
                    queue_num=st["qrr"] % 4,
                )
                st["qrr"] += 1
                # fold edge weight in: gt *= w (paired layout -> DVE 2x mode)
                st0 = int(p.sub_base[pp]) + gi * GSUB
                nsub = cnt // 128
                wap = wt_sb[:, st0:st0 + nsub, :]
                wb = AP(wap.tensor, wap.offset,
                        [wap.ap[0], [2, nsub], [0, HID // 2], [1, 2]])
                gtap = gt[:, :nsub, :]
                g4 = AP(gtap.tensor, gtap.offset,
                        [gtap.ap[0], [HID, nsub], [2, HID // 2], [1, 2]])
                nc.vector.tensor_tensor(g4, g4, wb, op=mybir.AluOpType.mult)
                gmap[(pp, gi)] = gt

            def ensure_gather(pp, gi):
                for d in range(8):           # prefetch to pool depth
                    emit_gather(pp, gi + d)
                return st["gmap"][(pp, gi)]

            def s_tile_for(sidx):
                s_tiles = st["s_tiles"]
                blk = sidx // SCHUNK
                if blk not in s_tiles:
                    for k in list(s_tiles):
                        if k < blk - 1:
                            del s_tiles[k]
                    s_load(blk)
                s_load(blk + 1)              # prefetch next block
                return s_tiles[blk], (sidx % SCHUNK) * 32

            def prefetch():
                for pp in range(n_pass):
                    ensure_idx(pp, 0)
                for b in range(4):
                    s_load(b)

            def run(layer, interleave_cb=None):
                sq_acc = small.tile([128, HID], FP32, tag="sqacc")
                sum_acc = small.tile([128, HID], FP32, tag="sumacc")
                nc.vector.memset(sq_acc[:], 0.0)
                nc.vector.memset(sum_acc[:], 0.0)
                mm = p.mm
                op_i = 0
                for pg in range(p.n_pair):
                    for q in range(quads):
                        if interleave_cb is not None:
                            interleave_cb(pg, q)
                        if pg == 0 and q == quads - 16:
                            for pp2 in (2, 3):
                                for g2 in range(3):
                                    emit_gather(pp2, g2)
                        psq = apsum.tile([128, HID], FP32, tag="agg")
                        while op_i < len(mm) and mm[op_i]["pg"] == pg \
                                and mm[op_i]["q"] == q:
                            op = mm[op_i]
                            pp, t = op["p"], op["t"]
                            gi, sub = t // GSUB, t % GSUB
                            gt = ensure_gather(pp, gi)
                            stile, scol = s_tile_for(op["s"])
                            nc.tensor.matmul(
                                psq[op["wi"] * 32:(op["wi"] + 1) * 32, :],
                                stile[:, scol:scol + 32],
                                gt[:, sub, :],
                                start=op_i in p.first_op,
                                stop=op_i in p.last_op,
                                tile_position=(0, op["wi"] * 32),
                                skip_group_check=True,
                            )
                            op_i += 1
                        hslot = hq(q)
                        if pg == 0:
                            nc.scalar.activation(hslot, psq[:],
                                                 mybir.ActivationFunctionType.Copy)
                        else:
                            # final pair: accumulate, then BN statistics
                            nc.vector.tensor_tensor(hslot, hslot,
                                                    psq[:], op=mybir.AluOpType.add)
                            sqt = fpool.tile([128, HID], FP32, tag="sqt")
                            nc.vector.tensor_tensor(sqt[:], hslot, hslot,
                                                    op=mybir.AluOpType.mult)
                            nc.vector.tensor_tensor(sq_acc[:], sq_acc[:], sqt[:],
                                                    op=mybir.AluOpType.add)
                            nc.vector.tensor_tensor(sum_acc[:], sum_acc[:],
                                                    hslot,
                                                    op=mybir.AluOpType.add)

                # stats: partition-reduce via ones matmul -> [1, HID] each
                pst = tpsum.tile([1, HID], FP32, tag="mmq", bufs=2)
                pst2 = tpsum.tile([1, HID], FP32, tag="mmq", bufs=2)
                nc.tensor.matmul(pst[:], ones_sb[:], sum_acc[:])
                nc.tensor.matmul(pst2[:], ones_sb[:], sq_acc[:])
                stat_loc = dram.tile([1, 2 * HID], FP32, name=f"stat_loc{layer}")
                stat_glob = dram.tile([1, 2 * HID], FP32, addr_space="Shared",
                                      name=f"stat_glob{layer}")
                st_sb = small.tile([1, 2, HID], FP32, tag="statsb")
                nc.vector.tensor_copy(st_sb[:, 0, :], pst[:])
                nc.vector.tensor_copy(st_sb[:, 1, :], pst2[:])
                nc.sync.dma_start(stat_loc[:], st_sb[:].opt())
                nc.gpsimd.collective_compute(
                    "AllReduce", mybir.AluOpType.add, replica_groups=groups,
                    ins=[stat_loc[:]], outs=[stat_glob[:]],
                )
                stg = small.tile([1, 2, HID], FP32, tag="statg")
                nc.sync.dma_start(stg[:].opt(), stat_glob[:])
                mu = small.tile([1, HID], FP32, tag="mu")
                nc.vector.tensor_scalar_mul(mu[:], stg[:, 0, :], 1.0 / N)
                var = small.tile([1, HID], FP32, tag="var")
                musq = small.tile([1, HID], FP32, tag="musq")
                nc.vector.tensor_tensor(musq[:], mu[:], mu[:],
                                        op=mybir.AluOpType.mult)
                nc.vector.tensor_scalar_mul(var[:], stg[:, 1, :], 1.0 / N)
                nc.vector.tensor_tensor(var[:], var[:], musq[:],
                                        op=mybir.AluOpType.subtract)
                nc.vector.tensor_scalar_add(var[:], var[:], float(p.EPS))
                sd = small.tile([1, HID], FP32, tag="sd")
                nc.scalar.activation(sd[:], var[:],
                                     mybir.ActivationFunctionType.Sqrt)
                rsd = small.tile([1, HID], FP32, tag="rsd")
                nc.vector.reciprocal(rsd[:], sd[:])
                gi_ = 0 if layer == 1 else 2
                sc = small.tile([1, HID], FP32, tag="sc")
                nc.vector.tensor_tensor(sc[:], rsd[:], gb_sb[:, gi_, :],
                                        op=mybir.AluOpType.mult)
                sh = small.tile([1, HID], FP32, tag="sh")
                nc.vector.tensor_tensor(sh[:], mu[:], sc[:],
                                        op=mybir.AluOpType.mult)
                nc.vector.tensor_tensor(sh[:], gb_sb[:, gi_ + 1, :], sh[:],
                                        op=mybir.AluOpType.subtract)
                sc_b = small.tile([128, HID], FP32, tag="scb")
                sh_b = small.tile([128, HID], FP32, tag="shb")
                nc.gpsimd.partition_broadcast(sc_b[:], sc[:])
                nc.gpsimd.partition_broadcast(sh_b[:], sh[:])
                return sc_b, sh_b

            class A:
                pass
            a = A()
            a.prefetch = prefetch
            a.run = run
            return a

        def bcast(t, n):
            # [128, 1] -> stride-0 broadcast AP [128, n]
            a = t[:]
            return AP(a.tensor, a.offset, [a.ap[0], [0, n]])

        def make_producer2(chunk_done):
            # t2 = h1 @ W2, h1 stored in h_pre (normalized in place)
            def produce_quad(q):
                pt = tpsum.tile([128, 128], FP16, tag="tr", bufs=2)
                nc.tensor.transpose(pt[:], hq(q), ident[:])
                h1T = fpool.tile([128, 128], FP16, tag="h1T")
                nc.vector.tensor_copy(h1T[:], pt[:])
                ps = tpsum.tile([128, HID], FP32, tag="mmq", bufs=2)
                nc.tensor.matmul(ps[:], h1T[:], W2_sb[:])
                o16 = fpool.tile([128, HID], FP16, tag="o16")
                nc.scalar.activation(o16[:], ps[:],
                                     mybir.ActivationFunctionType.Copy)
                i = pass_of(q)
                off = (q - p.qstart[i]) * 128
                nc.scalar.dma_start(shard2[i][off:off + 128, :], o16[:])
                chunk_done(q, shard2)
            return produce_quad

        # chunked AllGather: pass-major table layout makes each chunk's
        # output contiguous, so comm overlaps the producing matmul phase.
        bounds = {p.qstart[i] + nq - 1: i for i, nq in
                  enumerate([p.qstart[i + 1] - p.qstart[i] if i + 1 < n_pass
                             else quads - p.qstart[i] for i in range(n_pass)])}

        def make_chunk_done(table, skip_pass0=False):
            def chunk_done(q, shard):
                if q not in bounds:
                    return
                i = bounds[q]
                if skip_pass0 and i == 0:
                    return
                nc.gpsimd.collective_compute(
                    "AllGather", mybir.AluOpType.bypass, replica_groups=groups,
                    ins=[shard[i][:, :].opt()],
                    outs=[table[i][:, :].opt()],
                )
            return chunk_done

        # ---------- layer 1 ----------
        l1 = mk_agg(table1)
        l1.prefetch()
        prod1 = make_producer1(make_chunk_done(table1))
        for t in range(quads):
            prod1(t)
        def rep(t, k):
            # [128, HID] -> AP repeated k times along a stride-0 middle dim
            a = t[:]
            return AP(a.tensor, a.offset, [a.ap[0], [0, k], [1, HID]])

        def norm_chunks(i):
            # pass i's quads in batches of <=6 (big DVE/ACT ops)
            q0, q1 = p.qstart[i], qb_end[i]
            q = q0
            while q < q1:
                k = min(6, q1 - q)
                yield q, k
                q += k

        qb_end = [p.qstart[i + 1] if i + 1 < n_pass else quads
                  for i in range(n_pass)]

        sc_b, sh_b = l1.run(1)
        l2 = mk_agg(table2)
        l2.prefetch()
        # ---------- layer 2 ----------
        # normalize in multi-quad batches (stage-batched to avoid ACT FIFO
        # head-of-line blocking), then produce; per pass so the AllGather
        # and the first layer-2 gathers fire as early as possible.
        prod2 = make_producer2(make_chunk_done(table2))
        for i in range(n_pass):
            for q, k in norm_chunks(i):
                w = k * HID
                tmp = fpool.tile([128, 6 * HID], FP32, tag="ntmp")
                hsl = h_pre[i][:, q - p.qstart[i]:q - p.qstart[i] + k, :].opt()
                nc.vector.tensor_tensor(tmp[:, :w], hsl, rep(sc_b, k),
                                        op=mybir.AluOpType.mult)
                nc.vector.tensor_tensor(tmp[:, :w], tmp[:, :w], rep(sh_b, k),
                                        op=mybir.AluOpType.add)
                nc.scalar.activation(hsl, tmp[:, :w],
                                     mybir.ActivationFunctionType.Relu)
            for q in range(p.qstart[i], qb_end[i]):
                prod2(q)
        sc2, sh2 = l2.run(2)
        # normalize + relu -> output (node-major, fp16; host casts to fp32)
        for i in range(n_pass):
            for q, k in norm_chunks(i):
                w = k * HID
                tmp = fpool.tile([128, 6 * HID], FP32, tag="ntmp")
                hsl = h_pre[i][:, q - p.qstart[i]:q - p.qstart[i] + k, :].opt()
                nc.vector.tensor_tensor(tmp[:, :w], hsl, rep(sc2, k),
                                        op=mybir.AluOpType.mult)
                nc.vector.tensor_tensor(tmp[:, :w], tmp[:, :w], rep(sh2, k),
                                        op=mybir.AluOpType.add)
                ot = fpool.tile([128, 6 * HID], FP16, tag="otile")
                nc.scalar.activation(ot[:, :w], tmp[:, :w],
                                     mybir.ActivationFunctionType.Relu)
                eng = [nc.sync, nc.scalar][q % 2]
                eng.dma_start(out_t.ap()[q * 128:q * 128 + k * 128, :]
                              .rearrange("(k pp) h -> pp k h", pp=128),
                              ot[:, :w].rearrange("p (k h) -> p k h", h=HID))

    nc.compile()
    return nc


def make_inputs(p, x, W1, W2, g1, be1, g2, be2):
    """Per-core input maps."""
    D_IN = x.shape[1]
    in_maps = []
    ident = np.eye(128, dtype=np.float16)
    for c in range(p.n_cores):
        rows = p.perm_of[c * p.nsh:(c + 1) * p.nsh]
        xs = np.zeros((p.nsh, D_IN), dtype=np.float16)
        valid = rows >= 0
        xs[valid] = np.asarray(x)[rows[valid]].astype(np.float16)
        m = {
            "xT": np.ascontiguousarray(xs.T),
            "W1": np.asarray(W1, np.float16), "W2": np.asarray(W2, np.float16),
            "g1": np.asarray(g1, np.float32).reshape(1, -1),
            "be1": np.asarray(be1, np.float32).reshape(1, -1),
            "g2": np.asarray(g2, np.float32).reshape(1, -1),
            "be2": np.asarray(be2, np.float32).reshape(1, -1),
            "ident": ident,
            "S": np.ascontiguousarray(p.S[c]),
            "wt": np.ascontiguousarray(p.w_tiles[c]),
        }
        for pp in range(p.n_pass):
            m[f"idx{pp}"] = p.idx_wrapped[pp][c]
        in_maps.append(m)
    return in_maps


def assemble_output(p, results):
    out = np.zeros((p.N, p.HID), dtype=np.float32)
    for c in range(p.n_cores):
        rows = p.perm_of[c * p.nsh:(c + 1) * p.nsh]
        valid = rows >= 0
        out[rows[valid]] = results[c]["out"][valid].astype(np.float32)
    return out


# ---------------- public entry point ----------------
N_NODES = 100000
D_IN_C = 256
HID_C = 128
EPS_C = 1e-5
N_CORES = 8


def kernel(x, edge_index, edge_weight, W1, b1, g1, be1, W2, b2, g2, be2):
    """Full (unsharded) inputs -> full [N, HID] output, computed on 8 TRN2
    NeuronCores. b1/b2 are accepted but cancel exactly in training-mode
    BatchNorm (BN subtracts the batch mean, which contains the bias)."""
    from concourse.bass_utils import run_bass_kernel_spmd

    x = np.asarray(x, dtype=np.float32)
    edge_index = np.asarray(edge_index)
    edge_weight = np.asarray(edge_weight, dtype=np.float32)
    p = build_plan(edge_index, edge_weight, N_NODES, D_IN_C, HID_C, EPS_C,
                   n_cores=N_CORES)
    nc = build_nc(p)
    in_maps = make_inputs(p, x, W1, W2, g1, be1, g2, be2)
    res = run_bass_kernel_spmd(nc, in_maps, core_ids=list(range(N_CORES)))
    return assemble_output(p, res.results)



# revision 64
# speedup vs baseline: 1.1303x; 1.1303x over previous
"""GCN (2-layer, GCNConv+BN+ReLU) on 8 TRN2 NeuronCores via Bass.

Host plan: permute nodes so that pass p (= original src quarter) occupies
permuted rows [p*QROWS, (p+1)*QROWS); within each quarter the nodes are dealt
to 2 cores x WPC windows of 32 nodes, balancing in-degree. Edges are
dst-sharded; per (window, pass) chunk sizes form a COMMON schedule across
cores (SPMD: one program, per-core data).

Device per layer:
  t = h @ W (PE)  -> fp16 table shard -> AllGather -> full table
  4 gather streams (dma_gather, 2048 idx/call, 4 queues) -> msg tiles
  one DVE broadcast multiply folds the edge weight into each gathered tile
  staircase matmuls: S_onehot(fp8)[128s, 32win].T @ msg -> PSUM quad accumulate
  flush -> h_pre (fp16); BN stats (+AllReduce) -> normalize+ReLU
"""
import sys
sys.path.insert(0, '/opt/trn_rl_repo')

import numpy as np
import concourse.bass as bass
import concourse.bacc as bacc
import concourse.tile as tile
from concourse import mybir
from concourse.ap import AP
from contextlib import ExitStack

FP32 = mybir.dt.float32
FP16 = mybir.dt.float16
F8 = mybir.dt.float8e4
I16 = mybir.dt.int16

GSLOTS = 1024                 # slots per gather call (8 sub-tiles of 128)
GSUB = GSLOTS // 128


class Plan:
    pass


def build_plan(edge_index, edge_weight, N, D_IN, HID, EPS, n_cores=8, n_pass=4, win=32, seed=0):
    """Host-side schedule. Returns Plan with per-core tensors + common schedule.

    Table layout is pass-major: pass p (a contiguous range of quads in every
    core's shard) occupies table rows [tbase[p], tbase[p] + n_cores*crows[p]),
    core-major within the pass. Each pass's AllGather output is contiguous and
    its subtable is < 32768 rows (int16 gather indices)."""
    p = Plan()
    src = np.asarray(edge_index[0], dtype=np.int64)
    dst = np.asarray(edge_index[1], dtype=np.int64)
    w = np.asarray(edge_weight, dtype=np.float32)
    E = src.shape[0]

    nsh = ((N + n_cores - 1) // n_cores + 127) // 128 * 128   # 12544
    ntot = nsh * n_cores
    wpc = nsh // win                            # windows per core 392
    quads = nsh // 128                          # 98

    # quad -> pass chunks. Mildly uneven: a smaller pass 0 lets the first
    # AllGather chunk (and the first gathers) fire earlier without starving
    # the pair-0 gather stream the way a tiny pass 0 does.
    # Cap: 8 cores * 128 * nq <= 32768 rows per pass (int16 gather indices).
    if n_pass == 4 and quads == 98:
        qb = [20, 45, 71, 98]
    else:
        qb = [((c + 1) * quads) // n_pass for c in range(n_pass)]    # cumulative
    qstart = [0] + qb[:-1]
    nquads_p = [qb[i] - qstart[i] for i in range(n_pass)]
    crows = [128 * nq for nq in nquads_p]       # rows per core per pass
    tbase = np.concatenate([[0], np.cumsum([n_cores * r for r in crows])]).astype(np.int64)
    pass_of_quad = np.zeros(quads, dtype=np.int64)
    for i in range(n_pass):
        pass_of_quad[qstart[i]:qb[i]] = i
    p.qstart, p.crows, p.tbase = qstart, crows, tbase

    # ---- node -> (core, window, pos) balanced assignment (global snake) ----
    indeg = np.bincount(dst, minlength=N)
    order = np.argsort(-indeg, kind="stable")
    nbins = n_cores * wpc
    padded = np.full(nbins * win, -1, dtype=np.int64)
    padded[:N] = order
    grid = padded.reshape(win, nbins)
    for r in range(1, win, 2):
        grid[r] = grid[r][::-1]
    # bin b -> core b % n_cores, window b // n_cores
    bcore = np.arange(nbins) % n_cores
    bwin = np.arange(nbins) // n_cores
    # shard row (within core) of each grid cell
    cell_core = np.broadcast_to(bcore, (win, nbins))
    nodes = grid.reshape(-1)
    cc = cell_core.reshape(-1)
    valid = nodes >= 0

    def placement(wslot):
        slot_vec = wslot[bcore, bwin]
        cell_row = np.broadcast_to(slot_vec * win, (win, nbins)) \
            + np.arange(win)[:, None]
        cr = cell_row.reshape(-1)
        row_of = np.full(N, -1, dtype=np.int64)
        row_of[nodes[valid]] = cc[valid] * nsh + cr[valid]
        perm_of = np.full(ntot, -1, dtype=np.int64)
        perm_of[cc[valid] * nsh + cr[valid]] = nodes[valid]
        return row_of, perm_of

    # table row of a global shard row
    def table_row_of(grow):
        core = grow // nsh
        srow = grow % nsh
        qd = srow // 128
        pp = pass_of_quad[qd]
        return (tbase[pp] + core * np.asarray(crows)[pp]
                + (srow - 128 * np.asarray(qstart)[pp])), pp

    # ---- edges to permuted space ----
    def edge_stats(row_of):
        gdst = row_of[dst]
        gsrc = row_of[src]
        tsrc, epass = table_row_of(gsrc)
        lidx = tsrc - tbase[epass]
        assert lidx.max() < 32768
        ecore = gdst // nsh
        ewin = (gdst % nsh) // win
        key = (ecore * wpc + ewin) * n_pass + epass
        cnt = np.bincount(key, minlength=n_cores * wpc * n_pass) \
            .reshape(n_cores, wpc, n_pass)
        return gdst, epass, lidx, ecore, ewin, cnt

    # phase 1: identity placement, measure per-(window, pass) counts
    wslot = np.broadcast_to(np.arange(wpc), (n_cores, wpc)).copy()
    row_of, perm_of = placement(wslot)
    _, _, _, _, _, cnt = edge_stats(row_of)
    # phase 2: within each pass region, permute windows per core so count
    # vectors align across cores (schedule is max-over-cores). Window moves
    # inside a region keep every node's src-pass, so counts just permute.
    try:
        from scipy.optimize import linear_sum_assignment
        for r in range(n_pass):
            s0, s1 = 4 * qstart[r], 4 * qb[r]
            env = cnt[0, s0:s1].astype(np.float64)
            for c in range(1, n_cores):
                C = np.maximum(env[:, None, :], cnt[c, s0:s1][None, :, :]).sum(2) \
                    - env.sum(1)[:, None]
                ri, ci = linear_sum_assignment(C)
                wslot[c, s0 + ci] = s0 + np.arange(s1 - s0)
                env = np.maximum(env, cnt[c, s0:s1][ci])
        row_of, perm_of = placement(wslot)
    except Exception:
        pass
    gdst, epass, lidx, ecore, ewin, cnt = edge_stats(row_of)
    gsrc = row_of[src]
    sched = cnt.max(axis=0)             # [wpc, n_pass] common chunk sizes
    stream_len = sched.sum(axis=0)
    for pp in range(n_pass):
        extra = (-stream_len[pp]) % 128
        sched[wpc - 1, pp] += extra
    stream_len = sched.sum(axis=0)
    p.stream_len = stream_len.astype(np.int64)

    chunk_off = np.zeros((wpc + 1, n_pass), dtype=np.int64)
    chunk_off[1:] = np.cumsum(sched, axis=0)

    # ---- per-core slot arrays ----
    idx_arr = np.zeros((n_cores, n_pass, int(stream_len.max())), dtype=np.int16)
    w_arr = np.zeros((n_cores, n_pass, int(stream_len.max())), dtype=np.float32)
    col_arr = np.zeros((n_cores, n_pass, int(stream_len.max())), dtype=np.int32)
    eorder = np.lexsort((lidx, epass, ewin, ecore))
    po = epass[eorder]; co = ecore[eorder]
    wo = ewin[eorder]; li = lidx[eorder]; ww = w[eorder]
    gdo = gdst[eorder]
    ekey = (co * wpc + wo) * n_pass + po
    grp_start = np.zeros(n_cores * wpc * n_pass + 1, dtype=np.int64)
    grp_start[1:] = np.cumsum(cnt.reshape(-1))
    pos_in_grp = np.arange(E) - grp_start[ekey]
    slot = chunk_off[wo, po] + pos_in_grp
    idx_arr[co, po, slot] = li.astype(np.int16)
    w_arr[co, po, slot] = ww
    col_arr[co, po, slot] = (gdo % nsh) % 128

    p.N, p.E, p.n_cores, p.n_pass = N, E, n_cores, n_pass
    p.D_IN, p.HID, p.EPS = D_IN, HID, EPS
    p.nsh, p.ntot, p.win, p.wpc, p.quads = nsh, ntot, win, wpc, quads
    p.perm_of, p.row_of = perm_of, row_of
    p.sched, p.chunk_off = sched, chunk_off

    # layer-1 pass-0 table is computed locally on every core (x is cheap to
    # replicate for those rows) so the first gathers don't wait on AllGather.
    # x0_nodes[t] = original node id of table1[0] row t (or -1 for padding).
    crows0 = crows[0]
    x0_nodes = np.full(n_cores * crows0, -1, dtype=np.int64)
    for c in range(n_cores):
        rows = perm_of[c * nsh:c * nsh + crows0]
        x0_nodes[c * crows0:(c + 1) * crows0] = rows
    p.x0_nodes = x0_nodes
    p.crows0 = crows0

    # ---- matmul schedule (common, pass-PAIR-major) ----
    # Processing pass pairs (0,1) then (2,3) lets aggregation start as soon
    # as the first AllGather chunks land; PSUM accumulates within a pair and
    # partial sums are added into SBUF between pairs.
    mm = []
    s_count = 0
    n_pair = n_pass // 2
    for pg in range(n_pair):
        for q in range(quads):
            for wi in range(4):
                wdx = q * 4 + wi
                group_has_op = False
                for pp in (2 * pg, 2 * pg + 1):
                    a = int(chunk_off[wdx, pp]); b = int(chunk_off[wdx + 1, pp])
                    if b == a:
                        continue
                    group_has_op = True
                    t0, t1 = a // 128, (b - 1) // 128
                    for t in range(t0, t1 + 1):
                        mm.append(dict(q=q, wi=wi, p=pp, pg=pg, t=t, s=s_count,
                                       lo=max(a, t * 128), hi=min(b, (t + 1) * 128)))
                        s_count += 1
                if not group_has_op:
                    # zero-S placeholder so the PSUM strip gets start+stop
                    mm.append(dict(q=q, wi=wi, p=2 * pg, pg=pg, t=0, s=s_count,
                                   lo=0, hi=0))
                    s_count += 1
    p.n_pair = n_pair

    # ---- S one-hot fill (vectorized) + empty-op pruning ----
    op_lo = np.array([op["lo"] for op in mm])
    op_hi = np.array([op["hi"] for op in mm])
    op_p = np.array([op["p"] for op in mm])
    op_t = np.array([op["t"] for op in mm])
    op_wi = np.array([op["wi"] for op in mm])
    lens = op_hi - op_lo
    opidx = np.repeat(np.arange(s_count), lens)
    sl = np.concatenate([np.arange(a, b) for a, b in zip(op_lo, op_hi)]) if s_count else np.array([], np.int64)
    rows = sl - op_t[opidx] * 128
    # usage per (core, entry): valid one-hot hits
    ent_core = []
    ent_rows = []
    ent_cols = []
    ent_op = []
    op_used = np.zeros(s_count, dtype=bool)
    for c in range(n_cores):
        cols = col_arr[c, op_p[opidx], sl] - op_wi[opidx] * 32
        wv = w_arr[c, op_p[opidx], sl]
        m = (cols >= 0) & (cols < 32) & (wv != 0)
        ent_core.append(np.full(m.sum(), c))
        ent_rows.append(rows[m])
        ent_cols.append(cols[m])
        ent_op.append(opidx[m])
        np.logical_or.at(op_used, opidx[m], True)

    # keep at least one op per (q, wi, pair) group so every PSUM strip gets
    # start+stop within each pair phase
    group_of = {}
    for i, op in enumerate(mm):
        group_of.setdefault((op["q"], op["wi"], op["pg"]), []).append(i)
    for g, ops_in_g in group_of.items():
        if not op_used[ops_in_g].any():
            op_used[ops_in_g[0]] = True

    old2new = np.full(s_count, -1, dtype=np.int64)
    old2new[op_used] = np.arange(op_used.sum())
    mm = [dict(op, s=int(old2new[i])) for i, op in enumerate(mm) if op_used[i]]
    p.mm, p.n_mm = mm, int(op_used.sum())

    first_op = {}
    last_op = {}
    for i, op in enumerate(mm):
        k = (op["q"], op["wi"], op["pg"])
        if k not in first_op:
            first_op[k] = i
        last_op[k] = i
    p.first_op = {v: True for v in first_op.values()}
    p.last_op = {v: True for v in last_op.values()}

    S = np.zeros((n_cores, 128, 32 * p.n_mm), dtype=mybir.dt.np(F8))
    one = np.ones(1, dtype=mybir.dt.np(F8))[0]
    for c in range(n_cores):
        so = old2new[ent_op[c]]
        S[c, ent_rows[c], so * 32 + ent_cols[c]] = one
    p.S = S

    # w per slot, gather-tile aligned: [128, nst_total] fp16
    nst_per_pass = [int(stream_len[pp]) // 128 for pp in range(n_pass)]
    p.sub_base = np.concatenate([[0], np.cumsum(nst_per_pass)]).astype(np.int64)
    nst_total = int(p.sub_base[-1])
    p.nst_total = nst_total
    wt = np.zeros((n_cores, 128, nst_total, 2), dtype=np.float16)
    for c in range(n_cores):
        for pp in range(n_pass):
            L = int(stream_len[pp])
            v = w_arr[c, pp, :L].reshape(-1, 128).T.astype(np.float16)
            wt[c, :, p.sub_base[pp]:p.sub_base[pp + 1], 0] = v
            wt[c, :, p.sub_base[pp]:p.sub_base[pp + 1], 1] = v
    p.w_tiles = wt

    p.idx_wrapped = []
    for pp in range(n_pass):
        L = int(stream_len[pp])
        a = idx_arr[:, pp, :L]
        wr = np.stack([np.tile(a[c].reshape(-1, 16).T, (8, 1)) for c in range(n_cores)])
        p.idx_wrapped.append(np.ascontiguousarray(wr))

    p.gathers = []
    for pp in range(n_pass):
        L = int(stream_len[pp])
        assert L % 128 == 0
        p.gathers.append([(o, min(GSLOTS, L - o))
                          for o in range(0, L, GSLOTS)])
    return p


def build_nc(p, num_bufs=3):
    """Build the bass program from the common schedule."""
    n_pass, nsh, quads = p.n_pass, p.nsh, p.quads
    D_IN, HID = p.D_IN, p.HID
    din_t = D_IN // 128
    N = p.N

    nc = bacc.Bacc("TRN2", debug=False, num_devices=p.n_cores, num_swdge_queues=4)
    xT_in = nc.dram_tensor("xT", [D_IN, nsh], FP16, kind="ExternalInput")
    W1_in = nc.dram_tensor("W1", [D_IN, HID], FP16, kind="ExternalInput")
    W2_in = nc.dram_tensor("W2", [HID, HID], FP16, kind="ExternalInput")
    g1_in = nc.dram_tensor("g1", [1, HID], FP32, kind="ExternalInput")
    be1_in = nc.dram_tensor("be1", [1, HID], FP32, kind="ExternalInput")
    g2_in = nc.dram_tensor("g2", [1, HID], FP32, kind="ExternalInput")
    be2_in = nc.dram_tensor("be2", [1, HID], FP32, kind="ExternalInput")
    ident_in = nc.dram_tensor("ident", [128, 128], FP16, kind="ExternalInput")
    S_in = nc.dram_tensor("S", [128, 32 * p.n_mm], F8, kind="ExternalInput")
    wt_in = nc.dram_tensor("wt", [128, p.nst_total, 2], FP16, kind="ExternalInput")
    idx_ins = [nc.dram_tensor(f"idx{pp}", [128, int(p.stream_len[pp]) // 16], I16,
                              kind="ExternalInput") for pp in range(n_pass)]
    out_t = nc.dram_tensor("out", [nsh, HID], FP16, kind="ExternalOutput")

    groups = [list(range(p.n_cores))]

    with tile.TileContext(nc) as tc, ExitStack() as ctx:
        dram = ctx.enter_context(tc.tile_pool(name="dram", bufs=1, space="DRAM"))
        const = ctx.enter_context(tc.tile_pool(name="const", bufs=1))
        persist = ctx.enter_context(tc.tile_pool(name="persist", bufs=1))
        small = ctx.enter_context(tc.tile_pool(name="small", bufs=2))
        tpsum = ctx.enter_context(tc.tile_pool(name="tpsum", bufs=1, space="PSUM"))
        apsum = ctx.enter_context(tc.tile_pool(name="apsum", bufs=4, space="PSUM"))
        gpools = [ctx.enter_context(tc.tile_pool(name=f"gath{i}", bufs=8))
                  for i in range(n_pass)]
        ipools = [ctx.enter_context(tc.tile_pool(name=f"idxp{i}", bufs=2))
                  for i in range(n_pass)]
        spool = ctx.enter_context(tc.tile_pool(name="spool", bufs=4))
        fpool = ctx.enter_context(tc.tile_pool(name="flush", bufs=4))

        # resident xT first (needed by the very first matmuls)
        xT_sb = const.tile([128, din_t, nsh], FP16)
        XCH = 16
        xcw = nsh // XCH
        for i in range(XCH):
            eng = [nc.sync, nc.scalar][i % 2]
            eng.dma_start(
                xT_sb[:, :, i * xcw:(i + 1) * xcw],
                xT_in.ap()[:, i * xcw:(i + 1) * xcw]
                .rearrange("(a b) c -> b a c", b=128))
        W1_sb = const.tile([128, din_t, HID], FP16)
        nc.sync.dma_start(W1_sb[:], W1_in.ap().rearrange("(a b) c -> b a c", b=128))
        W2_sb = const.tile([128, HID], FP16)
        nc.scalar.dma_start(W2_sb[:], W2_in.ap())
        ident = const.tile([128, 128], FP16)
        nc.sync.dma_start(ident[:], ident_in.ap())
        ones_sb = const.tile([128, 1], FP32)
        nc.vector.memset(ones_sb[:], 1.0)
        gb_sb = const.tile([1, 4, HID], FP32)
        nc.sync.dma_start(gb_sb[:, 0, :], g1_in.ap())
        nc.sync.dma_start(gb_sb[:, 1, :], be1_in.ap())
        nc.sync.dma_start(gb_sb[:, 2, :], g2_in.ap())
        nc.sync.dma_start(gb_sb[:, 3, :], be2_in.ap())
        wt_sb = const.tile([128, p.nst_total, 2], FP16)
        nc.scalar.dma_start(wt_sb[:], wt_in.ap())

        # per-pass tiles: a single tile would serialize on coarse
        # write-after-read dependencies (pass p's AllGather read blocks
        # pass p+1's stores; layer-2 flushes block on producer2 reads).
        npq = [p.qstart[i + 1] - p.qstart[i] if i + 1 < n_pass
               else quads - p.qstart[i] for i in range(n_pass)]
        h_pre = [persist.tile([128, npq[i], HID], FP16, name=f"hpre{i}")
                 for i in range(n_pass)]

        def hq(q):
            for i in reversed(range(n_pass)):
                if q >= p.qstart[i]:
                    return h_pre[i][:, q - p.qstart[i], :]

        def pass_of(q):
            for i in reversed(range(n_pass)):
                if q >= p.qstart[i]:
                    return i

        # DRAM staging (per pass)
        shard1 = [dram.tile([p.crows[i], HID], FP16, name=f"sh1_{i}")
                  for i in range(n_pass)]
        shard2 = [dram.tile([p.crows[i], HID], FP16, name=f"sh2_{i}")
                  for i in range(n_pass)]
        table1 = [dram.tile([p.n_cores * p.crows[i], HID], FP16,
                            addr_space="Shared", name=f"tab1_{i}")
                  for i in range(n_pass)]
        table2 = [dram.tile([p.n_cores * p.crows[i], HID], FP16,
                            addr_space="Shared", name=f"tab2_{i}")
                  for i in range(n_pass)]

        def make_producer1(chunk_done):
            # t1 = x @ W1 from the SBUF-resident xT; cast+store on scalar
            def produce_quad(t):
                ps = tpsum.tile([128, HID], FP32, tag="mmq", bufs=2)
                for k in range(din_t):
                    nc.tensor.matmul(ps[:], xT_sb[:, k, t * 128:(t + 1) * 128],
                                     W1_sb[:, k, :],
                                     start=(k == 0), stop=(k == din_t - 1))
                o16 = fpool.tile([128, HID], FP16, tag="o16")
                nc.scalar.activation(o16[:], ps[:],
                                     mybir.ActivationFunctionType.Copy)
                i = pass_of(t)
                off = (t - p.qstart[i]) * 128
                nc.scalar.dma_start(shard1[i][off:off + 128, :], o16[:])
                chunk_done(t, shard1)
            return produce_quad

        SCHUNK = 64
        IB = 4                       # gather calls per idx-load batch

        def mk_agg(table):
            """Per-layer aggregation state: prefetch() then run(layer)."""
            st = {"gmap": {}, "imap": {}, "s_tiles": {}, "qrr": 0}

            def ensure_idx(pp, bi):
                if (pp, bi) in st["imap"]:
                    return st["imap"][(pp, bi)]
                lo = bi * IB * GSLOTS
                if lo >= int(p.stream_len[pp]):
                    return None
                it = ipools[pp].tile([128, IB * GSLOTS // 16], I16, tag=f"idx{pp}")
                hi = min(int(p.stream_len[pp]), lo + IB * GSLOTS)
                nc.sync.dma_start(it[:, :(hi - lo) // 16],
                                  idx_ins[pp].ap()[:, lo // 16:hi // 16])
                st["imap"][(pp, bi)] = it
                return it

            def s_load(blk):
                if blk in st["s_tiles"] or blk * SCHUNK >= p.n_mm:
                    return
                stile = spool.tile([128, SCHUNK * 32], F8, tag="S")
                lo = blk * SCHUNK * 32
                hi = min(32 * p.n_mm, lo + SCHUNK * 32)
                nc.sync.dma_start(stile[:, :hi - lo], S_in.ap()[:, lo:hi])
                st["s_tiles"][blk] = stile

            def emit_gather(pp, gi):
                gmap = st["gmap"]
                if (pp, gi) in gmap or gi >= len(p.gathers[pp]):
                    return
                off, cnt = p.gathers[pp][gi]
                it = ensure_idx(pp, gi // IB)
                ensure_idx(pp, gi // IB + 1)
                i0 = (gi % IB) * (GSLOTS // 16)
                gt = gpools[pp].tile([128, GSUB, HID], FP16, tag=f"g{pp}")
                nc.gpsimd.dma_gather(
                    gt[:, :cnt // 128, :],
                    table[pp][:, :],
                    it[:, i0:i0 + cnt // 16],
                    num_idxs=cnt, num_idxs_reg=cnt, elem_size=HID,
                    queue_num=st["qrr"] % 4,
                )
                st["qrr"] += 1
                # fold edge weight in: gt *= w (paired layout -> DVE 2x mode)
                st0 = int(p.sub_base[pp]) + gi * GSUB
                nsub = cnt // 128
                wap = wt_sb[:, st0:st0 + nsub, :]
                wb = AP(wap.tensor, wap.offset,
                        [wap.ap[0], [2, nsub], [0, HID // 2], [1, 2]])
                gtap = gt[:, :nsub, :]
                g4 = AP(gtap.tensor, gtap.offset,
                        [gtap.ap[0], [HID, nsub], [2, HID // 2], [1, 2]])
                nc.vector.tensor_tensor(g4, g4, wb, op=mybir.AluOpType.mult)
                gmap[(pp, gi)] = gt

            def ensure_gather(pp, gi):
                for d in range(6):           # prefetch five calls ahead
                    emit_gather(pp, gi + d)
                return st["gmap"][(pp, gi)]

            def s_tile_for(sidx):
                s_tiles = st["s_tiles"]
                blk = sidx // SCHUNK
                if blk not in s_tiles:
                    for k in list(s_tiles):
                        if k < blk - 1:
                            del s_tiles[k]
                    s_load(blk)
                s_load(blk + 1)              # prefetch next block
                return s_tiles[blk], (sidx % SCHUNK) * 32

            def prefetch():
                for pp in range(n_pass):
                    ensure_idx(pp, 0)
                for b in range(4):
                    s_load(b)

            def run(layer, interleave_cb=None):
                sq_acc = small.tile([128, HID], FP32, tag="sqacc")
                sum_acc = small.tile([128, HID], FP32, tag="sumacc")
                nc.vector.memset(sq_acc[:], 0.0)
                nc.vector.memset(sum_acc[:], 0.0)
                mm = p.mm
                op_i = 0
                for pg in range(p.n_pair):
                    for q in range(quads):
                        if interleave_cb is not None:
                            interleave_cb(pg, q)
                        if pg == 0 and q == quads - 16:
                            for pp2 in (2, 3):
                                for g2 in range(3):
                                    emit_gather(pp2, g2)
                        psq = apsum.tile([128, HID], FP32, tag="agg")
                        while op_i < len(mm) and mm[op_i]["pg"] == pg \
                                and mm[op_i]["q"] == q:
                            op = mm[op_i]
                            pp, t = op["p"], op["t"]
                            gi, sub = t // GSUB, t % GSUB
                            gt = ensure_gather(pp, gi)
                            stile, scol = s_tile_for(op["s"])
                            nc.tensor.matmul(
                                psq[op["wi"] * 32:(op["wi"] + 1) * 32, :],
                                stile[:, scol:scol + 32],
                                gt[:, sub, :],
                                start=op_i in p.first_op,
                                stop=op_i in p.last_op,
                                tile_position=(0, op["wi"] * 32),
                                skip_group_check=True,
                            )
                            op_i += 1
                        hslot = hq(q)
                        if pg == 0:
                            nc.scalar.activation(hslot, psq[:],
                                                 mybir.ActivationFunctionType.Copy)
                        else:
                            # final pair: accumulate, then BN statistics
                            nc.vector.tensor_tensor(hslot, hslot,
                                                    psq[:], op=mybir.AluOpType.add)
                            sqt = fpool.tile([128, HID], FP32, tag="sqt")
                            nc.vector.tensor_tensor(sqt[:], hslot, hslot,
                                                    op=mybir.AluOpType.mult)
                            nc.vector.tensor_tensor(sq_acc[:], sq_acc[:], sqt[:],
                                                    op=mybir.AluOpType.add)
                            nc.vector.tensor_tensor(sum_acc[:], sum_acc[:],
                                                    hslot,
                                                    op=mybir.AluOpType.add)

                # stats: partition-reduce via ones matmul -> [1, HID] each
                pst = tpsum.tile([1, HID], FP32, tag="mmq", bufs=2)
                pst2 = tpsum.tile([1, HID], FP32, tag="mmq", bufs=2)
                nc.tensor.matmul(pst[:], ones_sb[:], sum_acc[:])
                nc.tensor.matmul(pst2[:], ones_sb[:], sq_acc[:])
                stat_loc = dram.tile([1, 2 * HID], FP32, name=f"stat_loc{layer}")
                stat_glob = dram.tile([1, 2 * HID], FP32, addr_space="Shared",
                                      name=f"stat_glob{layer}")
                st_sb = small.tile([1, 2, HID], FP32, tag="statsb")
                nc.vector.tensor_copy(st_sb[:, 0, :], pst[:])
                nc.vector.tensor_copy(st_sb[:, 1, :], pst2[:])
                nc.sync.dma_start(stat_loc[:], st_sb[:].opt())
                nc.gpsimd.collective_compute(
                    "AllReduce", mybir.AluOpType.add, replica_groups=groups,
                    ins=[stat_loc[:]], outs=[stat_glob[:]],
                )
                stg = small.tile([1, 2, HID], FP32, tag="statg")
                nc.sync.dma_start(stg[:].opt(), stat_glob[:])
                mu = small.tile([1, HID], FP32, tag="mu")
                nc.vector.tensor_scalar_mul(mu[:], stg[:, 0, :], 1.0 / N)
                var = small.tile([1, HID], FP32, tag="var")
                musq = small.tile([1, HID], FP32, tag="musq")
                nc.vector.tensor_tensor(musq[:], mu[:], mu[:],
                                        op=mybir.AluOpType.mult)
                nc.vector.tensor_scalar_mul(var[:], stg[:, 1, :], 1.0 / N)
                nc.vector.tensor_tensor(var[:], var[:], musq[:],
                                        op=mybir.AluOpType.subtract)
                nc.vector.tensor_scalar_add(var[:], var[:], float(p.EPS))
                sd = small.tile([1, HID], FP32, tag="sd")
                nc.scalar.activation(sd[:], var[:],
                                     mybir.ActivationFunctionType.Sqrt)
                rsd = small.tile([1, HID], FP32, tag="rsd")
                nc.vector.reciprocal(rsd[:], sd[:])
                gi_ = 0 if layer == 1 else 2
                sc = small.tile([1, HID], FP32, tag="sc")
                nc.vector.tensor_tensor(sc[:], rsd[:], gb_sb[:, gi_, :],
                                        op=mybir.AluOpType.mult)
                sh = small.tile([1, HID], FP32, tag="sh")
                nc.vector.tensor_tensor(sh[:], mu[:], sc[:],
                                        op=mybir.AluOpType.mult)
                nc.vector.tensor_tensor(sh[:], gb_sb[:, gi_ + 1, :], sh[:],
                                        op=mybir.AluOpType.subtract)
                sc_b = small.tile([128, HID], FP32, tag="scb")
                sh_b = small.tile([128, HID], FP32, tag="shb")
                nc.gpsimd.partition_broadcast(sc_b[:], sc[:])
                nc.gpsimd.partition_broadcast(sh_b[:], sh[:])
                return sc_b, sh_b

            class A:
                pass
            a = A()
            a.prefetch = prefetch
            a.run = run
            return a

        def bcast(t, n):
            # [128, 1] -> stride-0 broadcast AP [128, n]
            a = t[:]
            return AP(a.tensor, a.offset, [a.ap[0], [0, n]])

        def make_producer2(chunk_done):
            # t2 = h1 @ W2, h1 stored in h_pre (normalized in place)
            def produce_quad(q):
                pt = tpsum.tile([128, 128], FP16, tag="tr", bufs=2)
                nc.tensor.transpose(pt[:], hq(q), ident[:])
                h1T = fpool.tile([128, 128], FP16, tag="h1T")
                nc.vector.tensor_copy(h1T[:], pt[:])
                ps = tpsum.tile([128, HID], FP32, tag="mmq", bufs=2)
                nc.tensor.matmul(ps[:], h1T[:], W2_sb[:])
                o16 = fpool.tile([128, HID], FP16, tag="o16")
                nc.scalar.activation(o16[:], ps[:],
                                     mybir.ActivationFunctionType.Copy)
                i = pass_of(q)
                off = (q - p.qstart[i]) * 128
                nc.scalar.dma_start(shard2[i][off:off + 128, :], o16[:])
                chunk_done(q, shard2)
            return produce_quad

        # chunked AllGather: pass-major table layout makes each chunk's
        # output contiguous, so comm overlaps the producing matmul phase.
        bounds = {p.qstart[i] + nq - 1: i for i, nq in
                  enumerate([p.qstart[i + 1] - p.qstart[i] if i + 1 < n_pass
                             else quads - p.qstart[i] for i in range(n_pass)])}

        def make_chunk_done(table, skip_pass0=False):
            def chunk_done(q, shard):
                if q not in bounds:
                    return
                i = bounds[q]
                if skip_pass0 and i == 0:
                    return
                nc.gpsimd.collective_compute(
                    "AllGather", mybir.AluOpType.bypass, replica_groups=groups,
                    ins=[shard[i][:, :].opt()],
                    outs=[table[i][:, :].opt()],
                )
            return chunk_done

        # ---------- layer 1 ----------
        l1 = mk_agg(table1)
        l1.prefetch()
        prod1 = make_producer1(make_chunk_done(table1))
        for t in range(quads):
            prod1(t)
        def rep(t, k):
            # [128, HID] -> AP repeated k times along a stride-0 middle dim
            a = t[:]
            return AP(a.tensor, a.offset, [a.ap[0], [0, k], [1, HID]])

        def norm_chunks(i):
            # pass i's quads in batches of <=6 (big DVE/ACT ops)
            q0, q1 = p.qstart[i], qb_end[i]
            q = q0
            while q < q1:
                k = min(6, q1 - q)
                yield q, k
                q += k

        qb_end = [p.qstart[i + 1] if i + 1 < n_pass else quads
                  for i in range(n_pass)]

        sc_b, sh_b = l1.run(1)
        l2 = mk_agg(table2)
        l2.prefetch()
        # ---------- layer 2 ----------
        # normalize in multi-quad batches (stage-batched to avoid ACT FIFO
        # head-of-line blocking), then produce; per pass so the AllGather
        # and the first layer-2 gathers fire as early as possible.
        prod2 = make_producer2(make_chunk_done(table2))
        for i in range(n_pass):
            for q, k in norm_chunks(i):
                w = k * HID
                tmp = fpool.tile([128, 6 * HID], FP32, tag="ntmp")
                hsl = h_pre[i][:, q - p.qstart[i]:q - p.qstart[i] + k, :].opt()
                nc.vector.tensor_tensor(tmp[:, :w], hsl, rep(sc_b, k),
                                        op=mybir.AluOpType.mult)
                nc.vector.tensor_tensor(tmp[:, :w], tmp[:, :w], rep(sh_b, k),
                                        op=mybir.AluOpType.add)
                nc.scalar.activation(hsl, tmp[:, :w],
                                     mybir.ActivationFunctionType.Relu)
            for q in range(p.qstart[i], qb_end[i]):
                prod2(q)
        sc2, sh2 = l2.run(2)
        # normalize + relu -> output (node-major, fp16; host casts to fp32)
        for i in range(n_pass):
            for q, k in norm_chunks(i):
                w = k * HID
                tmp = fpool.tile([128, 6 * HID], FP32, tag="ntmp")
                hsl = h_pre[i][:, q - p.qstart[i]:q - p.qstart[i] + k, :].opt()
                nc.vector.tensor_tensor(tmp[:, :w], hsl, rep(sc2, k),
                                        op=mybir.AluOpType.mult)
                nc.vector.tensor_tensor(tmp[:, :w], tmp[:, :w], rep(sh2, k),
                                        op=mybir.AluOpType.add)
                ot = fpool.tile([128, 6 * HID], FP16, tag="otile")
                nc.scalar.activation(ot[:, :w], tmp[:, :w],
                                     mybir.ActivationFunctionType.Relu)
                eng = [nc.sync, nc.scalar][q % 2]
                eng.dma_start(out_t.ap()[q * 128:q * 128 + k * 128, :]
                              .rearrange("(k pp) h -> pp k h", pp=128),
                              ot[:, :w].rearrange("p (k h) -> p k h", h=HID))

    nc.compile()
    return nc


def make_inputs(p, x, W1, W2, g1, be1, g2, be2):
    """Per-core input maps."""
    D_IN = x.shape[1]
    in_maps = []
    ident = np.eye(128, dtype=np.float16)
    for c in range(p.n_cores):
        rows = p.perm_of[c * p.nsh:(c + 1) * p.nsh]
        xs = np.zeros((p.nsh, D_IN), dtype=np.float16)
        valid = rows >= 0
        xs[valid] = np.asarray(x)[rows[valid]].astype(np.float16)
        m = {
            "xT": np.ascontiguousarray(xs.T),
            "W1": np.asarray(W1, np.float16), "W2": np.asarray(W2, np.float16),
            "g1": np.asarray(g1, np.float32).reshape(1, -1),
            "be1": np.asarray(be1, np.float32).reshape(1, -1),
            "g2": np.asarray(g2, np.float32).reshape(1, -1),
            "be2": np.asarray(be2, np.float32).reshape(1, -1),
            "ident": ident,
            "S": np.ascontiguousarray(p.S[c]),
            "wt": np.ascontiguousarray(p.w_tiles[c]),
        }
        for pp in range(p.n_pass):
            m[f"idx{pp}"] = p.idx_wrapped[pp][c]
        in_maps.append(m)
    return in_maps


def assemble_output(p, results):
    out = np.zeros((p.N, p.HID), dtype=np.float32)
    for c in range(p.n_cores):
        rows = p.perm_of[c * p.nsh:(c + 1) * p.nsh]
        valid = rows >= 0
        out[rows[valid]] = results[c]["out"][valid].astype(np.float32)
    return out


# ---------------- public entry point ----------------
N_NODES = 100000
D_IN_C = 256
HID_C = 128
EPS_C = 1e-5
N_CORES = 8


def kernel(x, edge_index, edge_weight, W1, b1, g1, be1, W2, b2, g2, be2):
    """Full (unsharded) inputs -> full [N, HID] output, computed on 8 TRN2
    NeuronCores. b1/b2 are accepted but cancel exactly in training-mode
    BatchNorm (BN subtracts the batch mean, which contains the bias)."""
    from concourse.bass_utils import run_bass_kernel_spmd

    x = np.asarray(x, dtype=np.float32)
    edge_index = np.asarray(edge_index)
    edge_weight = np.asarray(edge_weight, dtype=np.float32)
    p = build_plan(edge_index, edge_weight, N_NODES, D_IN_C, HID_C, EPS_C,
                   n_cores=N_CORES)
    nc = build_nc(p)
    in_maps = make_inputs(p, x, W1, W2, g1, be1, g2, be2)
    res = run_bass_kernel_spmd(nc, in_maps, core_ids=list(range(N_CORES)))
    return assemble_output(p, res.results)

